# revision 15
# baseline (speedup 1.0000x reference)
"""GQA attention block (B=2, N=2048, D=2048, H=16, KV=4) on 8 TRN2 NeuronCores.

Sharding: sequence-parallel with replicated weights. Core c handles batch
b = c//4, query rows [ (c%4)*512 : (c%4+1)*512 ).  Each core computes its
own Q/K/V projections + RoPE for its row block, AllGathers rope'd K and V,
runs full (non-causal, mask==ones) softmax attention for all 16 heads over
its 512 query rows, and applies the output projection, writing its row
slice of the final output (transposed as [f, n]; host transposes back).

v2 layout/dtype strategy:
  * Every matmul runs with bf16 operands (fp32 PSUM accumulation).  All the
    big operands (x, wq, wkv, wo, K/V payloads) are already bf16 on the
    wire, so bf16 matmuls add no quantization vs f32r-widened ones, and
    they skip all widening copies (and get the HW fast-weight-load path).
  * K and V for kv-group pair {0,1} are projected first and leave in ONE
    combined AllGather (CC_A) at ~30us; groups {2,3} follow (CC_B).  The
    attention runs in two 8-head waves gated on CC_A / CC_B, so the
    collectives hide under Q-projection + wave-A compute.
  * The softmax denominator is accumulated OFF the PE: DVE/Pool (alternate
    per head) sum the exp tiles elementwise, and a single [128,1] ones
    matmul per head does the final partition reduction (8k PE cycles vs
    131k for per-tile denominator matmuls).
  * exp runs on [128,1024] PSUM pairs (two score tiles per activation
    instruction) to halve the ACT per-instruction overhead.
  * The output projection is interleaved per wave: after wave A, partial
    out-proj (8 heads) accumulates into an SBUF f32 tile, filling PE idle
    while ACT drains wave-A exps and CC_B lands; wave B's out-proj fuses
    (psum + bias) + partial on DVE and streams the result out.
"""

import numpy as np
import ml_dtypes

from concourse import bacc, tile, mybir
from concourse import bass_utils

F32 = mybir.dt.float32
F32R = mybir.dt.float32r
F16 = mybir.dt.float16
BF16 = mybir.dt.bfloat16

P = 128
B, N, D = 2, 2048, 2048
H, HKV, HD = 16, 4, 128
NL = 512          # local query rows per core
ND = D // P       # 16 d-tiles
NKJ = N // P      # 16 key tiles
NFI = D // P      # 16 output-feature tiles
SCALE = 1.0 / np.sqrt(HD)
N_CORES = 8

_CACHE = {}


def _emit(nc, tc, ext, consts, x, single_core=False, stop_after=None):
    """Emit one full forward pass; all tile names prefixed with `x`."""
    (xt_ext, wq_ext, wkv01_ext, wkv23_ext, wo_ext, bias_ext, cos_ext,
     sin_ext, outt_ext) = ext
    (ones_kj_dram,) = consts

    with tc.tile_pool(name=f"{x}const", bufs=1) as cpool, \
         tc.tile_pool(name=f"{x}qr", bufs=1) as qrpool, \
         tc.tile_pool(name=f"{x}fix", bufs=1) as fxpool, \
         tc.tile_pool(name=f"{x}rope", bufs=5) as rpool, \
         tc.tile_pool(name=f"{x}dram", bufs=1, space="DRAM") as dpool, \
         nc.allow_low_precision("bf16 matmuls; accum f32"):

        ones_kj = cpool.tile([P, 1], F32R, name=f"{x}ones_kj", tag="ones_kj")
        cos_sb = cpool.tile([P, NL], F32, name=f"{x}cos_sb", tag="cos_sb")
        sin_sb = cpool.tile([P, NL], F32, name=f"{x}sin_sb", tag="sin_sb")
        bias_sb = cpool.tile([P, NFI], F32, name=f"{x}bias_sb", tag="bias_sb")

        # combined K+V payloads, one per kv-group pair
        ag_in = [dpool.tile([P, 2048], BF16, name=f"{x}ag{p}_in",
                            tag=f"ag{p}_in") for p in range(2)]
        ag_out = [dpool.tile([4, P, 2048], BF16, name=f"{x}ag{p}_out",
                             tag=f"ag{p}_out") for p in range(2)]

        nc.sync.dma_start(out=cos_sb[:], in_=cos_ext[:])
        nc.sync.dma_start(out=sin_sb[:], in_=sin_ext[:])
        nc.sync.dma_start(out=ones_kj[:],
                          in_=ones_kj_dram.ap().bitcast(F32R))
        nc.sync.dma_start(out=bias_sb[:], in_=bias_ext[:])

        def rope(dst, src_ps, nm):
            """dst[128,NL] (bf16) = rope(src_ps[PSUM f32 128,NL]).

            ACT evicts PSUM twice: straight (ev) and half-swapped with the
            second half negated (sw); DVE then does
            y = ev*[cos;cos] + sw*[sin;sin] (3 ops).
            """
            ev = rpool.tile([P, NL], F32, name=f"{x}{nm}_ev", tag="ropet")
            nc.scalar.copy(out=ev[:], in_=src_ps[:])
            sw = rpool.tile([P, NL], F32, name=f"{x}{nm}_sw", tag="ropet")
            nc.scalar.copy(out=sw[0:64, :], in_=src_ps[64:128, :])
            nc.scalar.mul(out=sw[64:128, :], in_=src_ps[0:64, :], mul=-1.0)
            t = rpool.tile([P, NL], F32, name=f"{x}{nm}_t", tag="ropet")
            nc.vector.tensor_tensor(out=t[:], in0=ev[:], in1=cos_sb[:],
                                    op=mybir.AluOpType.mult)
            u = rpool.tile([P, NL], F32, name=f"{x}{nm}_u", tag="ropet")
            nc.vector.tensor_tensor(out=u[:], in0=sw[:], in1=sin_sb[:],
                                    op=mybir.AluOpType.mult)
            nc.vector.tensor_tensor(out=dst[:], in0=t[:], in1=u[:],
                                    op=mybir.AluOpType.add)

        qr_sb = [qrpool.tile([P, NL], BF16, name=f"{x}qr{h}", tag=f"qr{h}")
                 for h in range(H)]

        # gathered K,V stay live through attention; one tile per group pair
        # so wave A never picks up a dependency on the CC_B-gated fetches.
        # kt layout [hd, (j, gi, key)]; vt layout [key, (j, gi, hd)]:
        # both use offset j*1024 + gi*512 + u*128 for key-tile kj=(j,u).
        with tc.tile_pool(name=f"{x}kv", bufs=1) as kvpool:
            kt_sb = [kvpool.tile([P, 4 * 1024], BF16, name=f"{x}kt{p}",
                                 tag=f"kt{p}") for p in range(2)]
            vt_sb = [kvpool.tile([P, 4 * 1024], BF16, name=f"{x}vt{p}",
                                 tag=f"vt{p}") for p in range(2)]

            with tc.tile_pool(name=f"{x}xt", bufs=1) as xpool, \
                 tc.tile_pool(name=f"{x}stage", bufs=1) as stpool:
                xt_sb = xpool.tile([P, ND * NL], BF16, name=f"{x}xt",
                                   tag="xt")

                def xs(dt):
                    return xt_sb[:, dt * NL:(dt + 1) * NL]

                kv_stage = [stpool.tile([P, 2048], BF16, name=f"{x}kvs{p}",
                                        tag=f"kvs{p}") for p in range(2)]

                # ---- K+V projection for one group pair + its AllGather ----
                with tc.tile_pool(name=f"{x}wkv", bufs=4) as kvwpool, \
                     tc.tile_pool(name=f"{x}ppkv", bufs=1,
                                  space="PSUM") as ppkv:
                    for pair in range(2):
                        wext = wkv01_ext if pair == 0 else wkv23_ext
                        kab = ppkv.tile([P, 2 * NL], F32, name=f"{x}kab{pair}",
                                        tag="kab", bufs=2)
                        # one full PSUM bank per key-subtile: a matmul
                        # start=True reset is bank-wide, so packing two
                        # 256-wide accumulations into one bank corrupts the
                        # first dt contribution of the earlier one.
                        vts = [ppkv.tile([P, NL], F32, name=f"{x}vab{pair}_{t}",
                                         tag="vab4", bufs=4) for t in range(4)]
                        for c in range(ND // 2):
                            if pair == 0:
                                # x chunk DMAs interleave with the first
                                # pair's weight chunks on the SP queue
                                nc.sync.dma_start(
                                    out=xt_sb[:, c * 1024:(c + 1) * 1024],
                                    in_=xt_ext[2 * c:2 * c + 2]
                                    .transpose([1, 0, 2]))
                            wkv = kvwpool.tile([P, 1024], BF16,
                                               name=f"{x}wkv{pair}_{c}",
                                               tag="wkv")
                            nc.sync.dma_start(
                                out=wkv[:],
                                in_=wext[2 * c:2 * c + 2].transpose([1, 0, 2]))
                            for i in range(2):
                                dt = 2 * c + i
                                for gi in range(2):
                                    nc.tensor.matmul(
                                        kab[:, gi * NL:(gi + 1) * NL],
                                        wkv[:, i * NL + gi * P:
                                            i * NL + (gi + 1) * P],
                                        xs(dt),
                                        start=(dt == 0), stop=(dt == ND - 1))
                                for t in range(4):
                                    nc.tensor.matmul(
                                        vts[t][:, 0:256],
                                        xs(dt)[:, t * P:(t + 1) * P],
                                        wkv[:, i * NL + 256:i * NL + NL],
                                        start=(dt == 0), stop=(dt == ND - 1))
                        # rope K -> stage cols [0:1024); V -> [1024:2048)
                        for gi in range(2):
                            rope(kv_stage[pair][:, gi * NL:(gi + 1) * NL],
                                 kab[:, gi * NL:(gi + 1) * NL],
                                 f"k{pair}_{gi}")
                        for t in range(4):
                            for gi in range(2):
                                nc.vector.tensor_copy(
                                    out=kv_stage[pair][
                                        :, 1024 + gi * NL + t * P:
                                        1024 + gi * NL + (t + 1) * P],
                                    in_=vts[t][:, gi * P:(gi + 1) * P])
                        nc.gpsimd.dma_start(out=ag_in[pair][:],
                                            in_=kv_stage[pair][:])
                        if single_core:
                            nc.gpsimd.dma_start(out=ag_out[pair][0],
                                                in_=ag_in[pair][:])
                        else:
                            nc.gpsimd.collective_compute(
                                "AllGather",
                                mybir.AluOpType.bypass,
                                ins=[ag_in[pair][:]],
                                outs=[ag_out[pair][:]],
                                replica_groups=[[0, 1, 2, 3], [4, 5, 6, 7]],
                            )

                # ---- Q projection + RoPE (overlaps the collectives) ----
                with tc.tile_pool(name=f"{x}wq", bufs=4) as wqpool, \
                     tc.tile_pool(name=f"{x}ppq", bufs=1,
                                  space="PSUM") as ppq:
                    for hg in range(4):
                        qa = ppq.tile([P, 2 * NL], F32, name=f"{x}qa{hg}",
                                      tag="qp", bufs=4)
                        qb = ppq.tile([P, 2 * NL], F32, name=f"{x}qb{hg}",
                                      tag="qp", bufs=4)
                        psq = [qa[:, 0:NL], qa[:, NL:2 * NL],
                               qb[:, 0:NL], qb[:, NL:2 * NL]]
                        for dp in range(ND // 2):
                            wb = wqpool.tile([P, 1024], BF16,
                                             name=f"{x}wqb{hg}_{dp}",
                                             tag="wqb")
                            nc.sync.dma_start(
                                out=wb[:],
                                in_=wq_ext[hg, 2 * dp:2 * dp + 2]
                                .transpose([1, 0, 2]))
                            for i in range(2):
                                dt = 2 * dp + i
                                for hh in range(4):
                                    nc.tensor.matmul(
                                        psq[hh][:],
                                        wb[:, i * NL + hh * P:
                                           i * NL + (hh + 1) * P],
                                        xs(dt),
                                        start=(dt == 0),
                                        stop=(dt == ND - 1))
                        for hh in range(4):
                            h = hg * 4 + hh
                            rope(qr_sb[h], psq[hh], f"q{h}")

            # ---- wo prefetch (SP), then gathered K/V fetches ----
            # wo is needed by the wave-A partial out-proj, so its DMAs go
            # ahead of the CC_B-gated fetches on the SP queue.
            wo_bf = []
            with tc.tile_pool(name=f"{x}wo", bufs=1) as wopool, \
                 tc.tile_pool(name=f"{x}exps", bufs=3) as epool, \
                 tc.tile_pool(name=f"{x}acc", bufs=2) as apool, \
                 tc.tile_pool(name=f"{x}no", bufs=1) as nopool, \
                 tc.tile_pool(name=f"{x}oacc", bufs=1) as oapool, \
                 tc.tile_pool(name=f"{x}outsb", bufs=2) as opool:
                for fi in range(NFI):
                    wt = wopool.tile([P, H * P], BF16, name=f"{x}wob{fi}",
                                     tag="wob", bufs=16)
                    # SWDGE (Pool) issue: cheap, and crucially NOT the SP
                    # HWDGE queue where the CC_B-gated pair-1 fetches live —
                    # a wo load stuck behind those stalls the whole PE queue
                    # via the scheduler's hoisted Ldweights.
                    nc.gpsimd.dma_start(out=wt[:], in_=wo_ext[fi])
                    wo_bf.append(wt)

                # fetch gathered K/V straight into place (bf16, no widen).
                # pair-0 fetches issue from the Pool queue (cheap dispatch,
                # free until wave A); pair-1 from SP so a CC_B wait never
                # blocks wave-A Pool work.
                for pair in range(2):
                    eng = nc.gpsimd if pair == 0 else nc.sync
                    for j in range(4):
                        jj = 0 if single_core else j
                        o = j * 1024
                        eng.dma_start(out=kt_sb[pair][:, o:o + 1024],
                                      in_=ag_out[pair][jj][:, 0:1024])
                        eng.dma_start(out=vt_sb[pair][:, o:o + 1024],
                                      in_=ag_out[pair][jj][:, 1024:2048])

                out_acc = oapool.tile([P, NFI * NL], F32, name=f"{x}oacc",
                                      tag="oacc")

                def kslice(g, kj):
                    j, u = divmod(kj, 4)
                    o = j * 1024 + (g % 2) * NL + u * P
                    return kt_sb[g // 2][:, o:o + P]

                def vslice(g, kj):
                    j, u = divmod(kj, 4)
                    o = j * 1024 + (g % 2) * NL + u * P
                    return vt_sb[g // 2][:, o:o + P]

                # ---- attention waves + interleaved out-projection ----
                no_sb = [None] * H
                with tc.tile_pool(name=f"{x}ppatt", bufs=1,
                                  space="PSUM") as pp:
                    for pair in range(2):
                        heads = [2 * pair + (hh % 2) + 4 * (hh // 2)
                                 for hh in range(8)]
                        for hi, h in enumerate(heads):
                            g = h % HKV
                            av_ps = pp.tile([P, NL], F32, name=f"{x}av{h}",
                                            tag="av", bufs=2)
                            acc = apool.tile([P, NL], F32R,
                                             name=f"{x}acc{h}", tag="acc")
                            deng = nc.vector
                            e_tiles = {}
                            for step in range(NKJ + 2):
                                if step < NKJ:
                                    kj = step
                                    s_ps = pp.tile([P, NL], F32,
                                                   name=f"{x}s{h}_{kj}",
                                                   tag="sc", bufs=4)
                                    nc.tensor.matmul(
                                        s_ps[:], kslice(g, kj), qr_sb[h][:],
                                        start=True, stop=True)
                                    e_sb = epool.tile([P, NL], BF16,
                                                      name=f"{x}e{h}_{kj}",
                                                      tag="exp", bufs=4)
                                    nc.scalar.activation(
                                        e_sb[:], s_ps[:],
                                        mybir.ActivationFunctionType.Exp,
                                        scale=float(SCALE))
                                    e_tiles[kj] = e_sb
                                if step >= 2:
                                    kj = step - 2
                                    e_sb = e_tiles.pop(kj)
                                    nc.tensor.matmul(
                                        av_ps[:], vslice(g, kj), e_sb[:],
                                        start=(kj == 0),
                                        stop=(kj == NKJ - 1))
                                    if kj == 0:
                                        deng.tensor_copy(
                                            out=acc[:], in_=e_sb[:])
                                    else:
                                        deng.tensor_tensor(
                                            out=acc[:], in0=acc[:],
                                            in1=e_sb[:],
                                            op=mybir.AluOpType.add)
                            den_ps = pp.tile([1, NL], F32, name=f"{x}den{h}",
                                             tag="den", bufs=1)
                            nc.tensor.matmul(den_ps[:], ones_kj[:], acc[:],
                                             start=True, stop=True)
                            recip = fxpool.tile([1, NL], F32,
                                                name=f"{x}rc{h}",
                                                tag="recip", bufs=2)
                            nc.vector.reciprocal(out=recip[:], in_=den_ps[:])
                            bc_sb = fxpool.tile([P, NL], F32,
                                                name=f"{x}bcs{h}",
                                                tag="bcs", bufs=2)
                            nc.gpsimd.partition_broadcast(bc_sb[:], recip[:])
                            no = nopool.tile([P, NL], BF16, name=f"{x}no{h}",
                                             tag=f"no{h}")
                            nc.vector.tensor_tensor(out=no[:], in0=av_ps[:],
                                                    in1=bc_sb[:],
                                                    op=mybir.AluOpType.mult)
                            no_sb[h] = no

                        if stop_after == "attn" and pair == 1:
                            nc.sync.dma_start(
                                out=outt_ext[0],
                                in_=no_sb[0][:].bitcast(BF16))
                            return

                        # partial out-projection over this wave's heads
                        for fi in range(NFI):
                            ps = pp.tile([P, NL], F32, name=f"{x}po{pair}_{fi}",
                                         tag="pso", bufs=1)
                            for h in heads:
                                nc.tensor.matmul(
                                    ps[:], wo_bf[fi][:, h * P:(h + 1) * P],
                                    no_sb[h][:],
                                    start=(h == heads[0]),
                                    stop=(h == heads[-1]))
                            oa = out_acc[:, fi * NL:(fi + 1) * NL]
                            if pair == 0:
                                nc.vector.tensor_copy(out=oa, in_=ps[:])
                            else:
                                o_sb = opool.tile([P, NL], BF16,
                                                  name=f"{x}o{fi}", tag="osb")
                                nc.vector.scalar_tensor_tensor(
                                    out=o_sb[:], in0=ps[:],
                                    scalar=bias_sb[:, fi:fi + 1],
                                    in1=oa,
                                    op0=mybir.AluOpType.add,
                                    op1=mybir.AluOpType.add)
                                nc.scalar.dma_start(out=outt_ext[fi],
                                                    in_=o_sb[:])


def build_program(reps=1, single_core=False):
    nc = bacc.Bacc("TRN2", target_bir_lowering=False, debug=False,
                   num_devices=1 if single_core else N_CORES)

    ext = (
        nc.dram_tensor("xt", [ND, P, NL], BF16,
                       kind="ExternalInput").ap(),
        nc.dram_tensor("wqtt", [4, ND, P, NL], BF16,
                       kind="ExternalInput").ap(),
        nc.dram_tensor("wkv01t", [ND, P, NL], BF16,
                       kind="ExternalInput").ap(),
        nc.dram_tensor("wkv23t", [ND, P, NL], BF16,
                       kind="ExternalInput").ap(),
        nc.dram_tensor("wott", [NFI, P, H * P], BF16,
                       kind="ExternalInput").ap(),
        nc.dram_tensor("biast", [P, NFI], F32, kind="ExternalInput").ap(),
        nc.dram_tensor("cost", [P, NL], F32, kind="ExternalInput").ap(),
        nc.dram_tensor("sint", [P, NL], F32, kind="ExternalInput").ap(),
        nc.dram_tensor("outt", [NFI, P, NL], BF16,
                       kind="ExternalOutput").ap(),
    )
    consts = (
        nc.inline_tensor(np.ones((P, 1), np.float32), name="ones_kj_c"),
    )

    with tile.TileContext(nc) as tc:
        for r in range(reps):
            _emit(nc, tc, ext, consts, f"r{r}_" if reps > 1 else "",
                  single_core=single_core)

    nc.compile()
    return nc


def shard_inputs(x, cos, sin, wq, wkv, wo_w, wo_b):
    """Host-side prep: transpose/tile everything into DMA-friendly layouts."""
    x = np.asarray(x, np.float32)
    cos = np.asarray(cos, np.float32)
    sin = np.asarray(sin, np.float32)
    wq = np.asarray(wq, np.float32)
    wkv = np.asarray(wkv, np.float32)
    wo_w = np.asarray(wo_w, np.float32)
    wo_b = np.asarray(wo_b, np.float32)

    wqT = np.ascontiguousarray(wq.T)                      # [d, e]
    # tiles [hg, dt, 128, 512]
    wqtt = np.ascontiguousarray(
        wqT.reshape(ND, P, 4, NL).transpose(2, 0, 1, 3)).astype(
            ml_dtypes.bfloat16)
    wkvT = wkv.T                                          # [d, 1024]
    wk, wv = wkvT[:, 0:512], wkvT[:, 512:1024]
    # per pair: [d, 512] = [K pair (2*128) | V pair (2*128)]
    wkv01 = np.ascontiguousarray(
        np.concatenate([wk[:, 0:256], wv[:, 0:256]], axis=1)
    ).reshape(ND, P, NL).astype(ml_dtypes.bfloat16)
    wkv23 = np.ascontiguousarray(
        np.concatenate([wk[:, 256:512], wv[:, 256:512]], axis=1)
    ).reshape(ND, P, NL).astype(ml_dtypes.bfloat16)
    woT = wo_w.T                                          # [e, f]
    # [fi, a, h, b]: per fi a contiguous [128, 2048] block
    wott = np.ascontiguousarray(
        woT.reshape(H, P, NFI, P).transpose(2, 1, 0, 3)
    ).reshape(NFI, P, H * P).astype(ml_dtypes.bfloat16)
    biast = np.ascontiguousarray(wo_b.reshape(NFI, P).T)  # [128, 16] f32

    in_maps = []
    for c in range(N_CORES):
        b, blk = divmod(c, 4)
        r0 = blk * NL
        xt = np.ascontiguousarray(x[b, r0:r0 + NL, :].T).reshape(
            ND, P, NL).astype(ml_dtypes.bfloat16)
        cosT = cos[0, r0:r0 + NL, 0, :].T                 # [64, n]
        sinT = sin[0, r0:r0 + NL, 0, :].T
        cost = np.ascontiguousarray(np.vstack([cosT, cosT]))   # [128, n]
        sint = np.ascontiguousarray(np.vstack([sinT, sinT]))
        in_maps.append({
            "xt": xt, "wqtt": wqtt, "wkv01t": wkv01, "wkv23t": wkv23,
            "wott": wott, "biast": biast, "cost": cost, "sint": sint,
        })
    return in_maps


def assemble_output(results):
    out = np.empty((B, N, D), np.float32)
    for c in range(N_CORES):
        b, blk = divmod(c, 4)
        r0 = blk * NL
        # outt [NFI, P, NL] -> [d, n] -> transpose
        out[b, r0:r0 + NL, :] = results[c]["outt"].reshape(
            D, NL).astype(np.float32).T
    return out


def get_program(reps=1):
    key = ("nc", reps)
    if key not in _CACHE:
        _CACHE[key] = build_program(reps)
    return _CACHE[key]


def kernel(x, cos, sin, attn_mask, wq, wkv, wo_w, wo_b):
    # attn_mask is all-ones by construction (fill spec); ignored.
    nc = get_program()
    in_maps = shard_inputs(x, cos, sin, wq, wkv, wo_w, wo_b)
    res = bass_utils.run_bass_kernel_spmd(
        nc, in_maps, core_ids=list(range(N_CORES)))
    return assemble_output(res.results)


# revision 22
# speedup vs baseline: 1.0656x; 1.0656x over previous
"""GQA attention block (B=2, N=2048, D=2048, H=16, KV=4) on 8 TRN2 NeuronCores.

Sharding: sequence-parallel with replicated weights. Core c handles batch
b = c//4, query rows [ (c%4)*512 : (c%4+1)*512 ).  Each core computes its
own Q/K/V projections + RoPE for its row block, AllGathers rope'd K and V,
runs full (non-causal, mask==ones) softmax attention for all 16 heads over
its 512 query rows, and applies the output projection, writing its row
slice of the final output (transposed as [f, n]; host transposes back).

All matmuls run in f32r (a 16-bit matmul emits a separate InstLdweights per
matmul, which costs real PE time on HW even though the cost model says 0).
Everything big crosses the wire as bf16 and is widened to f32r on-chip
(x on DVE, wkv on Pool, wq/wo/K on ACT, V on Pool).

v3 schedule (vs. the original one-gather-per-tensor structure):
  * K and V for kv-group pair {0,1} are projected first and leave in ONE
    combined AllGather (CC_A) at ~30us; groups {2,3} follow (CC_B).  The
    attention runs in two 8-head waves gated on CC_A / CC_B so the
    collectives hide under Q-projection + wave-A compute.  (The collective
    cost is ~15us constant + bytes/BW, so two big combined gathers beat
    four small ones.)
  * The softmax denominator is mostly OFF the PE: kj % 4 == 0 tiles fold
    into a PSUM accumulation via [128,1] ones-matmuls, the other 12 tiles
    accumulate elementwise on DVE, and one final ones-matmul folds the DVE
    accumulator in.  Balances PE ~7.9us / DVE ~8.4us / ACT ~8.0us per head
    instead of 10.2us/head all-PE.
  * exp runs on [128,1024] PSUM pairs (two score tiles per activation
    instruction) to halve the ACT per-instruction overhead.
  * wo prefetch issues from the Pool SWDGE queue: on the SP HWDGE queue the
    scheduler parks it behind the CC_B-gated pair-1 fetches, and the PE's
    hoisted outproj Ldweights then stalls the whole PE queue on it.
  * V-projection PSUM tiles get a full bank per key-subtile: matmul
    start=True resets the whole 2KB bank, so two 256-wide accumulations
    in one bank corrupt each other.
"""

import numpy as np
import ml_dtypes

from concourse import bacc, tile, mybir
from concourse import bass_utils

F32 = mybir.dt.float32
F32R = mybir.dt.float32r
F16 = mybir.dt.float16
BF16 = mybir.dt.bfloat16

P = 128
B, N, D = 2, 2048, 2048
H, HKV, HD = 16, 4, 128
NL = 512          # local query rows per core
ND = D // P       # 16 d-tiles
NKJ = N // P      # 16 key tiles
NFI = D // P      # 16 output-feature tiles
SCALE = 1.0 / np.sqrt(HD)
N_CORES = 8

_CACHE = {}


def _emit(nc, tc, ext, consts, x, single_core=False, stop_after=None):
    """Emit one full forward pass; all tile names prefixed with `x`."""
    (xt_ext, wq_ext, wkv01_ext, wkv23_ext, wo_ext, bias_ext, cos_ext,
     sin_ext, outt_ext) = ext
    (ones_kj_dram,) = consts

    with tc.tile_pool(name=f"{x}const", bufs=1) as cpool, \
         tc.tile_pool(name=f"{x}qr", bufs=1) as qrpool, \
         tc.tile_pool(name=f"{x}fix", bufs=1) as fxpool, \
         tc.tile_pool(name=f"{x}rope", bufs=4) as rpool, \
         tc.tile_pool(name=f"{x}dram", bufs=1, space="DRAM") as dpool, \
         nc.allow_low_precision("f32r matmuls; accum f32"):

        ones_kj = cpool.tile([P, 1], F32R, name=f"{x}ones_kj", tag="ones_kj")
        cos_sb = cpool.tile([P, NL], F32, name=f"{x}cos_sb", tag="cos_sb")
        sin_sb = cpool.tile([P, NL], F32, name=f"{x}sin_sb", tag="sin_sb")
        bias_sb = cpool.tile([P, NFI], F32, name=f"{x}bias_sb", tag="bias_sb")

        # combined K+V payloads, one per kv-group pair
        ag_in = [dpool.tile([P, 2048], BF16, name=f"{x}ag{p}_in",
                            tag=f"ag{p}_in") for p in range(2)]
        ag_out = [dpool.tile([4, P, 2048], BF16, name=f"{x}ag{p}_out",
                             tag=f"ag{p}_out") for p in range(2)]

        nc.sync.dma_start(out=cos_sb[:], in_=cos_ext[:])
        nc.sync.dma_start(out=sin_sb[:], in_=sin_ext[:])
        nc.sync.dma_start(out=ones_kj[:],
                          in_=ones_kj_dram.ap().bitcast(F32R))
        nc.sync.dma_start(out=bias_sb[:], in_=bias_ext[:])

        def rope(dst, src_ps, nm):
            """dst[128,NL] = rope(src_ps[PSUM f32 128,NL]).

            ACT evicts PSUM twice: straight (ev) and half-swapped with the
            second half negated (sw); DVE then does
            y = ev*[cos;cos] + sw*[sin;sin] (3 ops).
            """
            ev = rpool.tile([P, NL], F32, name=f"{x}{nm}_ev", tag="ropet")
            nc.scalar.copy(out=ev[:], in_=src_ps[:])
            sw = rpool.tile([P, NL], F32, name=f"{x}{nm}_sw", tag="ropet")
            nc.scalar.copy(out=sw[0:64, :], in_=src_ps[64:128, :])
            nc.scalar.mul(out=sw[64:128, :], in_=src_ps[0:64, :], mul=-1.0)
            t = rpool.tile([P, NL], F32, name=f"{x}{nm}_t", tag="ropet")
            nc.vector.tensor_tensor(out=t[:], in0=ev[:], in1=cos_sb[:],
                                    op=mybir.AluOpType.mult)
            u = rpool.tile([P, NL], F32, name=f"{x}{nm}_u", tag="ropet")
            nc.vector.tensor_tensor(out=u[:], in0=sw[:], in1=sin_sb[:],
                                    op=mybir.AluOpType.mult)
            nc.vector.tensor_tensor(out=dst[:], in0=t[:], in1=u[:],
                                    op=mybir.AluOpType.add)

        qr_sb = [qrpool.tile([P, NL], F32R, name=f"{x}qr{h}", tag=f"qr{h}")
                 for h in range(H)]

        # gathered K,V stay live through attention; one tile per group pair
        # so wave A never picks up a dependency on the CC_B-gated fetches.
        # kt layout [hd, (j, gi, key)]; vt layout [key, (j, gi, hd)]:
        # both use offset j*1024 + gi*512 + u*128 for key-tile kj=(j,u).
        with tc.tile_pool(name=f"{x}kv", bufs=1) as kvpool:
            kt_sb = [kvpool.tile([P, 4 * 1024], F32R, name=f"{x}kt{p}",
                                 tag=f"kt{p}") for p in range(2)]
            vt_sb = [kvpool.tile([P, 4 * 1024], F32R, name=f"{x}vt{p}",
                                 tag=f"vt{p}") for p in range(2)]

            with tc.tile_pool(name=f"{x}xt", bufs=1) as xpool, \
                 tc.tile_pool(name=f"{x}stage", bufs=1) as stpool:
                xt_sb = xpool.tile([P, ND * NL], F32R, name=f"{x}xt",
                                   tag="xt")

                def xs(dt):
                    return xt_sb[:, dt * NL:(dt + 1) * NL]

                kv_stage = [stpool.tile([P, 2048], BF16, name=f"{x}kvs{p}",
                                        tag=f"kvs{p}") for p in range(2)]

                # ---- K+V projection per group pair + its AllGather ----
                with tc.tile_pool(name=f"{x}xb", bufs=3) as xbpool, \
                     tc.tile_pool(name=f"{x}wkv", bufs=3) as kvwpool, \
                     tc.tile_pool(name=f"{x}ppkv", bufs=1,
                                  space="PSUM") as ppkv:
                    for pair in range(2):
                        wext = wkv01_ext if pair == 0 else wkv23_ext
                        kab = ppkv.tile([P, 2 * NL], F32,
                                        name=f"{x}kab{pair}", tag="kab",
                                        bufs=2)
                        # one full PSUM bank per key-subtile: a matmul
                        # start=True reset is bank-wide, so packing two
                        # 256-wide accumulations into one bank corrupts
                        # the earlier one's first dt contribution.
                        vts = [ppkv.tile([P, NL], F32,
                                         name=f"{x}vab{pair}_{t}",
                                         tag="vab4", bufs=4)
                               for t in range(4)]
                        for c in range(ND // 2):
                            if pair == 0:
                                xb = xbpool.tile([P, 1024], BF16,
                                                 name=f"{x}xb{c}", tag="xb")
                                nc.sync.dma_start(
                                    out=xb[:],
                                    in_=xt_ext[2 * c:2 * c + 2]
                                    .transpose([1, 0, 2]))
                                nc.vector.tensor_copy(
                                    out=xt_sb[:, c * 1024:(c + 1) * 1024],
                                    in_=xb[:])
                            wkvb = kvwpool.tile([P, 1024], BF16,
                                                name=f"{x}wkvb{pair}_{c}",
                                                tag="wkvb")
                            nc.sync.dma_start(
                                out=wkvb[:],
                                in_=wext[2 * c:2 * c + 2].transpose([1, 0, 2]))
                            wkv = kvwpool.tile([P, 1024], F32R,
                                               name=f"{x}wkv{pair}_{c}",
                                               tag="wkv")
                            nc.gpsimd.tensor_copy(out=wkv[:], in_=wkvb[:])
                            for i in range(2):
                                dt = 2 * c + i
                                for gi in range(2):
                                    nc.tensor.matmul(
                                        kab[:, gi * NL:(gi + 1) * NL],
                                        wkv[:, i * NL + gi * P:
                                            i * NL + (gi + 1) * P],
                                        xs(dt),
                                        start=(dt == 0), stop=(dt == ND - 1))
                                for t in range(4):
                                    nc.tensor.matmul(
                                        vts[t][:, 0:256],
                                        xs(dt)[:, t * P:(t + 1) * P],
                                        wkv[:, i * NL + 256:i * NL + NL],
                                        start=(dt == 0), stop=(dt == ND - 1))
                        # rope K -> stage cols [0:1024); V -> [1024:2048)
                        for gi in range(2):
                            rope(kv_stage[pair][:, gi * NL:(gi + 1) * NL],
                                 kab[:, gi * NL:(gi + 1) * NL],
                                 f"k{pair}_{gi}")
                        for t in range(4):
                            for gi in range(2):
                                nc.vector.tensor_copy(
                                    out=kv_stage[pair][
                                        :, 1024 + gi * NL + t * P:
                                        1024 + gi * NL + (t + 1) * P],
                                    in_=vts[t][:, gi * P:(gi + 1) * P])
                        nc.gpsimd.dma_start(out=ag_in[pair][:],
                                            in_=kv_stage[pair][:])
                        if single_core:
                            nc.gpsimd.dma_start(out=ag_out[pair][0],
                                                in_=ag_in[pair][:])
                        else:
                            nc.gpsimd.collective_compute(
                                "AllGather",
                                mybir.AluOpType.bypass,
                                ins=[ag_in[pair][:]],
                                outs=[ag_out[pair][:]],
                                replica_groups=[[0, 1, 2, 3], [4, 5, 6, 7]],
                            )

                # ---- Q projection + RoPE (overlaps the collectives) ----
                with tc.tile_pool(name=f"{x}wq", bufs=3) as wqpool, \
                     tc.tile_pool(name=f"{x}ppq", bufs=1,
                                  space="PSUM") as ppq:
                    for hg in range(4):
                        qa = ppq.tile([P, 2 * NL], F32, name=f"{x}qa{hg}",
                                      tag="qp", bufs=4)
                        qb = ppq.tile([P, 2 * NL], F32, name=f"{x}qb{hg}",
                                      tag="qp", bufs=4)
                        psq = [qa[:, 0:NL], qa[:, NL:2 * NL],
                               qb[:, 0:NL], qb[:, NL:2 * NL]]
                        for dp in range(ND // 2):
                            wb = wqpool.tile([P, 1024], BF16,
                                             name=f"{x}wqb{hg}_{dp}",
                                             tag="wqb", bufs=4)
                            nc.sync.dma_start(
                                out=wb[:],
                                in_=wq_ext[hg, 2 * dp:2 * dp + 2]
                                .transpose([1, 0, 2]))
                            wt = wqpool.tile([P, 1024], F32R,
                                             name=f"{x}wq{hg}_{dp}",
                                             tag="wq")
                            nc.scalar.copy(out=wt[:], in_=wb[:])
                            for i in range(2):
                                dt = 2 * dp + i
                                for hh in range(4):
                                    nc.tensor.matmul(
                                        psq[hh][:],
                                        wt[:, i * NL + hh * P:
                                           i * NL + (hh + 1) * P],
                                        xs(dt),
                                        start=(dt == 0),
                                        stop=(dt == ND - 1))
                        for hh in range(4):
                            h = hg * 4 + hh
                            rope(qr_sb[h], psq[hh], f"q{h}")

            # ---- wo prefetch (Pool SWDGE) + gathered K/V fetch/widen ----
            wo_bf = []
            with tc.tile_pool(name=f"{x}wo", bufs=1) as wopool, \
                 tc.tile_pool(name=f"{x}exps", bufs=3) as epool, \
                 tc.tile_pool(name=f"{x}acc", bufs=2) as apool, \
                 tc.tile_pool(name=f"{x}no", bufs=1) as nopool, \
                 tc.tile_pool(name=f"{x}outsb", bufs=2) as opool:

                # fetch gathered K/V with SWDGE cast-DMAs (bf16 in DRAM
                # -> f32r in SBUF): no staging tiles, no widen ops.  Pair 0
                # is fetched before the attention; pair 1 is emitted BETWEEN
                # the waves so its CC_B wait at the Pool queue head only
                # blocks wave-B work.
                def fetch_pair(pair):
                    for j in range(4):
                        jj = 0 if single_core else j
                        o = j * 1024
                        nc.gpsimd.dma_start(
                            out=kt_sb[pair][:, o:o + 1024],
                            in_=ag_out[pair][jj][:, 0:1024])
                        nc.gpsimd.dma_start(
                            out=vt_sb[pair][:, o:o + 1024],
                            in_=ag_out[pair][jj][:, 1024:2048])

                fetch_pair(0)

                for fi in range(NFI):
                    wt = wopool.tile([P, H * P], BF16, name=f"{x}wob{fi}",
                                     tag="wob", bufs=6)
                    nc.gpsimd.dma_start(out=wt[:], in_=wo_ext[fi])
                    wo_bf.append(wt)

                def kslice(g, kj):
                    j, u = divmod(kj, 4)
                    o = j * 1024 + (g % 2) * NL + u * P
                    return kt_sb[g // 2][:, o:o + P]

                def vslice(g, kj):
                    j, u = divmod(kj, 4)
                    o = j * 1024 + (g % 2) * NL + u * P
                    return vt_sb[g // 2][:, o:o + P]

                # ---- attention: two 8-head waves ----
                no_sb = [None] * H
                with tc.tile_pool(name=f"{x}ppatt", bufs=1,
                                  space="PSUM") as pp:
                    for pair in range(2):
                        if pair == 1:
                            fetch_pair(1)
                        heads = [2 * pair + (hh % 2) + 4 * (hh // 2)
                                 for hh in range(8)]
                        for hi, h in enumerate(heads):
                            g = h % HKV
                            av_ps = pp.tile([P, NL], F32, name=f"{x}av{h}",
                                            tag="av", bufs=2)
                            den_ps = pp.tile([1, NL], F32, name=f"{x}den{h}",
                                             tag="den", bufs=2)
                            acc = apool.tile([P, NL], F32R,
                                             name=f"{x}acc{h}", tag="acc")
                            e_tiles = {}
                            for step in range(9):
                                if step < 8:
                                    s_ps = pp.tile([P, 2 * NL], F32,
                                                   name=f"{x}s{h}_{step}",
                                                   tag="sc", bufs=2)
                                    for i in range(2):
                                        kj = 2 * step + i
                                        nc.tensor.matmul(
                                            s_ps[:, i * NL:(i + 1) * NL],
                                            kslice(g, kj), qr_sb[h][:],
                                            start=True, stop=True)
                                    e_sb = epool.tile([P, 2 * NL], F32R,
                                                      name=f"{x}e{h}_{step}",
                                                      tag="exp")
                                    nc.scalar.activation(
                                        e_sb[:], s_ps[:],
                                        mybir.ActivationFunctionType.Exp,
                                        scale=float(SCALE))
                                    e_tiles[step] = e_sb
                                if step >= 1:
                                    p2 = step - 1
                                    e_sb = e_tiles.pop(p2)
                                    for i in range(2):
                                        kj = 2 * p2 + i
                                        esl = e_sb[:, i * NL:(i + 1) * NL]
                                        nc.tensor.matmul(
                                            av_ps[:], vslice(g, kj), esl,
                                            start=(kj == 0),
                                            stop=(kj == NKJ - 1))
                                        # denominator: 5 tiles fold in
                                        # on the PE; the other 11 accumulate
                                        # on DVE (folded in by a final mm).
                                        if kj % 3 == 0 and kj != 15:
                                            nc.tensor.matmul(
                                                den_ps[:], ones_kj[:], esl,
                                                start=(kj == 0), stop=False)
                                        elif kj == 1:
                                            nc.vector.tensor_copy(
                                                out=acc[:], in_=esl)
                                        else:
                                            nc.vector.tensor_tensor(
                                                out=acc[:], in0=acc[:],
                                                in1=esl,
                                                op=mybir.AluOpType.add)
                            nc.tensor.matmul(den_ps[:], ones_kj[:], acc[:],
                                             start=False, stop=True)
                            recip = fxpool.tile([1, NL], F32,
                                                name=f"{x}rc{h}",
                                                tag="recip", bufs=2)
                            nc.vector.reciprocal(out=recip[:], in_=den_ps[:])
                            bc_sb = fxpool.tile([P, NL], F32,
                                                name=f"{x}bcs{h}",
                                                tag="bcs", bufs=2)
                            nc.gpsimd.partition_broadcast(bc_sb[:],
                                                          recip[:])
                            no = nopool.tile([P, NL], F32R, name=f"{x}no{h}",
                                             tag=f"no{h}")
                            nc.vector.tensor_tensor(out=no[:], in0=av_ps[:],
                                                    in1=bc_sb[:],
                                                    op=mybir.AluOpType.mult)
                            no_sb[h] = no

                if stop_after == "attn":
                    nc.sync.dma_start(out=outt_ext[0],
                                      in_=no_sb[0][:].bitcast(F32))
                    return

                # ---- output projection (outT layout [f, n]) + bias ----
                with tc.tile_pool(name=f"{x}ppout", bufs=1,
                                  space="PSUM") as ppo:
                    for fi in range(NFI):
                        wo_sb = wopool.tile([P, H * P], F32R,
                                            name=f"{x}wo{fi}", tag="wo",
                                            bufs=2)
                        nc.scalar.copy(out=wo_sb[:], in_=wo_bf[fi][:])
                        o_sb = opool.tile([P, NL], BF16, name=f"{x}o{fi}",
                                          tag="osb")
                        ps = ppo.tile([P, NL], F32, name=f"{x}pso{fi}",
                                      tag="mm", bufs=2)
                        for h in range(H):
                            nc.tensor.matmul(
                                ps[:], wo_sb[:, h * P:(h + 1) * P],
                                no_sb[h][:],
                                start=(h == 0), stop=(h == H - 1))
                        nc.vector.tensor_scalar(
                            out=o_sb[:], in0=ps[:],
                            scalar1=bias_sb[:, fi:fi + 1],
                            scalar2=None, op0=mybir.AluOpType.add)
                        nc.scalar.dma_start(out=outt_ext[fi], in_=o_sb[:])


def build_program(reps=1, single_core=False):
    nc = bacc.Bacc("TRN2", target_bir_lowering=False, debug=False,
                   num_devices=1 if single_core else N_CORES)

    ext = (
        nc.dram_tensor("xt", [ND, P, NL], BF16,
                       kind="ExternalInput").ap(),
        nc.dram_tensor("wqtt", [4, ND, P, NL], BF16,
                       kind="ExternalInput").ap(),
        nc.dram_tensor("wkv01t", [ND, P, NL], BF16,
                       kind="ExternalInput").ap(),
        nc.dram_tensor("wkv23t", [ND, P, NL], BF16,
                       kind="ExternalInput").ap(),
        nc.dram_tensor("wott", [NFI, P, H * P], BF16,
                       kind="ExternalInput").ap(),
        nc.dram_tensor("biast", [P, NFI], F32, kind="ExternalInput").ap(),
        nc.dram_tensor("cost", [P, NL], F32, kind="ExternalInput").ap(),
        nc.dram_tensor("sint", [P, NL], F32, kind="ExternalInput").ap(),
        nc.dram_tensor("outt", [NFI, P, NL], BF16,
                       kind="ExternalOutput").ap(),
    )
    consts = (
        nc.inline_tensor(np.ones((P, 1), np.float32), name="ones_kj_c"),
    )

    with tile.TileContext(nc) as tc:
        for r in range(reps):
            _emit(nc, tc, ext, consts, f"r{r}_" if reps > 1 else "",
                  single_core=single_core)

    nc.compile()
    return nc


def shard_inputs(x, cos, sin, wq, wkv, wo_w, wo_b):
    """Host-side prep: transpose/tile everything into DMA-friendly layouts."""
    x = np.asarray(x, np.float32)
    cos = np.asarray(cos, np.float32)
    sin = np.asarray(sin, np.float32)
    wq = np.asarray(wq, np.float32)
    wkv = np.asarray(wkv, np.float32)
    wo_w = np.asarray(wo_w, np.float32)
    wo_b = np.asarray(wo_b, np.float32)

    wqT = np.ascontiguousarray(wq.T)                      # [d, e]
    # tiles [hg, dt, 128, 512]
    wqtt = np.ascontiguousarray(
        wqT.reshape(ND, P, 4, NL).transpose(2, 0, 1, 3)).astype(
            ml_dtypes.bfloat16)
    wkvT = wkv.T                                          # [d, 1024]
    wk, wv = wkvT[:, 0:512], wkvT[:, 512:1024]
    # per pair: [d, 512] = [K pair (2*128) | V pair (2*128)]
    wkv01 = np.ascontiguousarray(
        np.concatenate([wk[:, 0:256], wv[:, 0:256]], axis=1)
    ).reshape(ND, P, NL).astype(ml_dtypes.bfloat16)
    wkv23 = np.ascontiguousarray(
        np.concatenate([wk[:, 256:512], wv[:, 256:512]], axis=1)
    ).reshape(ND, P, NL).astype(ml_dtypes.bfloat16)
    woT = wo_w.T                                          # [e, f]
    # [fi, a, h, b]: per fi a contiguous [128, 2048] block
    wott = np.ascontiguousarray(
        woT.reshape(H, P, NFI, P).transpose(2, 1, 0, 3)
    ).reshape(NFI, P, H * P).astype(ml_dtypes.bfloat16)
    biast = np.ascontiguousarray(wo_b.reshape(NFI, P).T)  # [128, 16] f32

    in_maps = []
    for c in range(N_CORES):
        b, blk = divmod(c, 4)
        r0 = blk * NL
        xt = np.ascontiguousarray(x[b, r0:r0 + NL, :].T).reshape(
            ND, P, NL).astype(ml_dtypes.bfloat16)
        cosT = cos[0, r0:r0 + NL, 0, :].T                 # [64, n]
        sinT = sin[0, r0:r0 + NL, 0, :].T
        cost = np.ascontiguousarray(np.vstack([cosT, cosT]))   # [128, n]
        sint = np.ascontiguousarray(np.vstack([sinT, sinT]))
        in_maps.append({
            "xt": xt, "wqtt": wqtt, "wkv01t": wkv01, "wkv23t": wkv23,
            "wott": wott, "biast": biast, "cost": cost, "sint": sint,
        })
    return in_maps


def assemble_output(results):
    out = np.empty((B, N, D), np.float32)
    for c in range(N_CORES):
        b, blk = divmod(c, 4)
        r0 = blk * NL
        # outt [NFI, P, NL] -> [d, n] -> transpose
        out[b, r0:r0 + NL, :] = results[c]["outt"].reshape(
            D, NL).astype(np.float32).T
    return out


def get_program(reps=1):
    key = ("nc", reps)
    if key not in _CACHE:
        _CACHE[key] = build_program(reps)
    return _CACHE[key]


def kernel(x, cos, sin, attn_mask, wq, wkv, wo_w, wo_b):
    # attn_mask is all-ones by construction (fill spec); ignored.
    nc = get_program()
    in_maps = shard_inputs(x, cos, sin, wq, wkv, wo_w, wo_b)
    res = bass_utils.run_bass_kernel_spmd(
        nc, in_maps, core_ids=list(range(N_CORES)))
    return assemble_output(res.results)


# revision 24
# speedup vs baseline: 1.1064x; 1.0384x over previous
"""GQA attention block (B=2, N=2048, D=2048, H=16, KV=4) on 8 TRN2 NeuronCores.

Sharding: sequence-parallel with replicated weights. Core c handles batch
b = c//4, query rows [ (c%4)*512 : (c%4+1)*512 ).  Each core computes its
own Q/K/V projections + RoPE for its row block, AllGathers rope'd K and V,
runs full (non-causal, mask==ones) softmax attention for all 16 heads over
its 512 query rows, and applies the output projection, writing its row
slice of the final output (transposed as [f, n]; host transposes back).

All matmuls run in f32r (a 16-bit matmul emits a separate InstLdweights per
matmul, which costs real PE time on HW even though the cost model says 0).
Everything big crosses the wire as bf16 and is widened to f32r on-chip
(x on DVE, wkv on Pool, wq/wo/K on ACT, V on Pool).

v3 schedule (vs. the original one-gather-per-tensor structure):
  * K and V for kv-group pair {0,1} are projected first and leave in ONE
    combined AllGather (CC_A) at ~30us; groups {2,3} follow (CC_B).  The
    attention runs in two 8-head waves gated on CC_A / CC_B so the
    collectives hide under Q-projection + wave-A compute.  (The collective
    cost is ~15us constant + bytes/BW, so two big combined gathers beat
    four small ones.)
  * The softmax denominator is mostly OFF the PE: kj % 4 == 0 tiles fold
    into a PSUM accumulation via [128,1] ones-matmuls, the other 12 tiles
    accumulate elementwise on DVE, and one final ones-matmul folds the DVE
    accumulator in.  Balances PE ~7.9us / DVE ~8.4us / ACT ~8.0us per head
    instead of 10.2us/head all-PE.
  * exp runs on [128,1024] PSUM pairs (two score tiles per activation
    instruction) to halve the ACT per-instruction overhead.
  * wo prefetch issues from the Pool SWDGE queue: on the SP HWDGE queue the
    scheduler parks it behind the CC_B-gated pair-1 fetches, and the PE's
    hoisted outproj Ldweights then stalls the whole PE queue on it.
  * V-projection PSUM tiles get a full bank per key-subtile: matmul
    start=True resets the whole 2KB bank, so two 256-wide accumulations
    in one bank corrupt each other.
"""

import numpy as np
import ml_dtypes

from concourse import bacc, tile, mybir
from concourse import bass_utils

F32 = mybir.dt.float32
F32R = mybir.dt.float32r
F16 = mybir.dt.float16
BF16 = mybir.dt.bfloat16

P = 128
B, N, D = 2, 2048, 2048
H, HKV, HD = 16, 4, 128
NL = 512          # local query rows per core
ND = D // P       # 16 d-tiles
NKJ = N // P      # 16 key tiles
NFI = D // P      # 16 output-feature tiles
SCALE = 1.0 / np.sqrt(HD)
N_CORES = 8

_CACHE = {}


def _emit(nc, tc, ext, consts, x, single_core=False, stop_after=None):
    """Emit one full forward pass; all tile names prefixed with `x`."""
    (xt_ext, wq_ext, wkv01_ext, wkv23_ext, wo_ext, bias_ext, cos_ext,
     sin_ext, outt_ext) = ext
    (ones_kj_dram,) = consts

    with tc.tile_pool(name=f"{x}const", bufs=1) as cpool, \
         tc.tile_pool(name=f"{x}qr", bufs=1) as qrpool, \
         tc.tile_pool(name=f"{x}fix", bufs=1) as fxpool, \
         tc.tile_pool(name=f"{x}rope", bufs=5) as rpool, \
         tc.tile_pool(name=f"{x}dram", bufs=1, space="DRAM") as dpool, \
         nc.allow_low_precision("f32r matmuls; accum f32"):

        ones_kj = cpool.tile([P, 1], F16, name=f"{x}ones_kj", tag="ones_kj")
        cos_sb = cpool.tile([P, NL], F32, name=f"{x}cos_sb", tag="cos_sb")
        sin_sb = cpool.tile([P, NL], F32, name=f"{x}sin_sb", tag="sin_sb")
        bias_sb = cpool.tile([P, NFI], F32, name=f"{x}bias_sb", tag="bias_sb")

        # combined K+V payloads, one per kv-group pair
        ag_in = [dpool.tile([P, 2048], BF16, name=f"{x}ag{p}_in",
                            tag=f"ag{p}_in") for p in range(2)]
        ag_out = [dpool.tile([4, P, 2048], BF16, name=f"{x}ag{p}_out",
                             tag=f"ag{p}_out") for p in range(2)]

        nc.sync.dma_start(out=cos_sb[:], in_=cos_ext[:])
        nc.sync.dma_start(out=sin_sb[:], in_=sin_ext[:])
        nc.sync.dma_start(out=ones_kj[:], in_=ones_kj_dram.ap())
        nc.sync.dma_start(out=bias_sb[:], in_=bias_ext[:])

        def rope(dst, src_ps, nm):
            """dst[128,NL] = rope(src_ps[PSUM f32 128,NL]).

            ACT evicts PSUM twice: straight (ev) and half-swapped with the
            second half negated (sw); DVE then does
            y = ev*[cos;cos] + sw*[sin;sin] (3 ops).
            """
            ev = rpool.tile([P, NL], F32, name=f"{x}{nm}_ev", tag="ropet")
            nc.scalar.copy(out=ev[:], in_=src_ps[:])
            sw = rpool.tile([P, NL], F32, name=f"{x}{nm}_sw", tag="ropet")
            nc.scalar.copy(out=sw[0:64, :], in_=src_ps[64:128, :])
            nc.scalar.mul(out=sw[64:128, :], in_=src_ps[0:64, :], mul=-1.0)
            t = rpool.tile([P, NL], F32, name=f"{x}{nm}_t", tag="ropet")
            nc.vector.tensor_tensor(out=t[:], in0=ev[:], in1=cos_sb[:],
                                    op=mybir.AluOpType.mult)
            u = rpool.tile([P, NL], F32, name=f"{x}{nm}_u", tag="ropet")
            nc.vector.tensor_tensor(out=u[:], in0=sw[:], in1=sin_sb[:],
                                    op=mybir.AluOpType.mult)
            nc.vector.tensor_tensor(out=dst[:], in0=t[:], in1=u[:],
                                    op=mybir.AluOpType.add)

        qr_sb = [qrpool.tile([P, NL], BF16, name=f"{x}qr{h}", tag=f"qr{h}")
                 for h in range(H)]

        # gathered K,V stay live through attention; one tile per group pair
        # so wave A never picks up a dependency on the CC_B-gated fetches.
        # kt layout [hd, (j, gi, key)]; vt layout [key, (j, gi, hd)]:
        # both use offset j*1024 + gi*512 + u*128 for key-tile kj=(j,u).
        with tc.tile_pool(name=f"{x}kv", bufs=1) as kvpool:
            kt_sb = [kvpool.tile([P, 4 * 1024], BF16, name=f"{x}kt{p}",
                                 tag=f"kt{p}") for p in range(2)]
            vt_sb = [kvpool.tile([P, 4 * 1024], BF16, name=f"{x}vt{p}",
                                 tag=f"vt{p}") for p in range(2)]

            with tc.tile_pool(name=f"{x}xt", bufs=1) as xpool, \
                 tc.tile_pool(name=f"{x}stage", bufs=1) as stpool:
                xt_sb = xpool.tile([P, ND * NL], BF16, name=f"{x}xt",
                                   tag="xt")

                def xs(dt):
                    return xt_sb[:, dt * NL:(dt + 1) * NL]

                kv_stage = [stpool.tile([P, 2048], BF16, name=f"{x}kvs{p}",
                                        tag=f"kvs{p}") for p in range(2)]

                # ---- K+V projection per group pair + its AllGather ----
                with tc.tile_pool(name=f"{x}wkv", bufs=3) as kvwpool, \
                     tc.tile_pool(name=f"{x}ppkv", bufs=1,
                                  space="PSUM") as ppkv:
                    for pair in range(2):
                        wext = wkv01_ext if pair == 0 else wkv23_ext
                        kab = ppkv.tile([P, 2 * NL], F32,
                                        name=f"{x}kab{pair}", tag="kab",
                                        bufs=2)
                        # one full PSUM bank per key-subtile: a matmul
                        # start=True reset is bank-wide, so packing two
                        # 256-wide accumulations into one bank corrupts
                        # the earlier one's first dt contribution.
                        vts = [ppkv.tile([P, NL], F32,
                                         name=f"{x}vab{pair}_{t}",
                                         tag="vab4", bufs=4)
                               for t in range(4)]
                        for c in range(ND // 2):
                            if pair == 0:
                                nc.sync.dma_start(
                                    out=xt_sb[:, c * 1024:(c + 1) * 1024],
                                    in_=xt_ext[2 * c:2 * c + 2]
                                    .transpose([1, 0, 2]))
                            wkv = kvwpool.tile([P, 1024], BF16,
                                               name=f"{x}wkv{pair}_{c}",
                                               tag="wkv")
                            nc.sync.dma_start(
                                out=wkv[:],
                                in_=wext[2 * c:2 * c + 2].transpose([1, 0, 2]))
                            for i in range(2):
                                dt = 2 * c + i
                                for gi in range(2):
                                    nc.tensor.matmul(
                                        kab[:, gi * NL:(gi + 1) * NL],
                                        wkv[:, i * NL + gi * P:
                                            i * NL + (gi + 1) * P],
                                        xs(dt),
                                        start=(dt == 0), stop=(dt == ND - 1))
                                for t in range(4):
                                    nc.tensor.matmul(
                                        vts[t][:, 0:256],
                                        xs(dt)[:, t * P:(t + 1) * P],
                                        wkv[:, i * NL + 256:i * NL + NL],
                                        start=(dt == 0), stop=(dt == ND - 1))
                        # rope K -> stage cols [0:1024); V -> [1024:2048)
                        for gi in range(2):
                            rope(kv_stage[pair][:, gi * NL:(gi + 1) * NL],
                                 kab[:, gi * NL:(gi + 1) * NL],
                                 f"k{pair}_{gi}")
                        for t in range(4):
                            for gi in range(2):
                                nc.vector.tensor_copy(
                                    out=kv_stage[pair][
                                        :, 1024 + gi * NL + t * P:
                                        1024 + gi * NL + (t + 1) * P],
                                    in_=vts[t][:, gi * P:(gi + 1) * P])
                        nc.gpsimd.dma_start(out=ag_in[pair][:],
                                            in_=kv_stage[pair][:])
                        if single_core:
                            nc.gpsimd.dma_start(out=ag_out[pair][0],
                                                in_=ag_in[pair][:])
                        else:
                            nc.gpsimd.collective_compute(
                                "AllGather",
                                mybir.AluOpType.bypass,
                                ins=[ag_in[pair][:]],
                                outs=[ag_out[pair][:]],
                                replica_groups=[[0, 1, 2, 3], [4, 5, 6, 7]],
                            )

                # ---- Q projection + RoPE (overlaps the collectives) ----
                with tc.tile_pool(name=f"{x}wq", bufs=3) as wqpool, \
                     tc.tile_pool(name=f"{x}ppq", bufs=1,
                                  space="PSUM") as ppq:
                    for hg in range(4):
                        qa = ppq.tile([P, 2 * NL], F32, name=f"{x}qa{hg}",
                                      tag="qp", bufs=4)
                        qb = ppq.tile([P, 2 * NL], F32, name=f"{x}qb{hg}",
                                      tag="qp", bufs=4)
                        psq = [qa[:, 0:NL], qa[:, NL:2 * NL],
                               qb[:, 0:NL], qb[:, NL:2 * NL]]
                        for dp in range(ND // 2):
                            wt = wqpool.tile([P, 1024], BF16,
                                             name=f"{x}wqb{hg}_{dp}",
                                             tag="wqb", bufs=4)
                            nc.sync.dma_start(
                                out=wt[:],
                                in_=wq_ext[hg, 2 * dp:2 * dp + 2]
                                .transpose([1, 0, 2]))
                            for i in range(2):
                                dt = 2 * dp + i
                                for hh in range(4):
                                    nc.tensor.matmul(
                                        psq[hh][:],
                                        wt[:, i * NL + hh * P:
                                           i * NL + (hh + 1) * P],
                                        xs(dt),
                                        start=(dt == 0),
                                        stop=(dt == ND - 1))
                        for hh in range(4):
                            h = hg * 4 + hh
                            rope(qr_sb[h], psq[hh], f"q{h}")

            # ---- wo prefetch (Pool SWDGE) + gathered K/V fetch/widen ----
            wo_bf = []
            with tc.tile_pool(name=f"{x}wo", bufs=1) as wopool, \
                 tc.tile_pool(name=f"{x}exps", bufs=3) as epool, \
                 tc.tile_pool(name=f"{x}acc", bufs=2) as apool, \
                 tc.tile_pool(name=f"{x}no", bufs=1) as nopool, \
                 tc.tile_pool(name=f"{x}oacc", bufs=1) as oapool, \
                 tc.tile_pool(name=f"{x}outsb", bufs=2) as opool:

                # fetch gathered K/V with SWDGE cast-DMAs (bf16 in DRAM
                # -> f32r in SBUF): no staging tiles, no widen ops.  Pair 0
                # is fetched before the attention; pair 1 is emitted BETWEEN
                # the waves so its CC_B wait at the Pool queue head only
                # blocks wave-B work.
                def fetch_pair(pair):
                    for j in range(4):
                        jj = 0 if single_core else j
                        o = j * 1024
                        nc.gpsimd.dma_start(
                            out=kt_sb[pair][:, o:o + 1024],
                            in_=ag_out[pair][jj][:, 0:1024])
                        nc.gpsimd.dma_start(
                            out=vt_sb[pair][:, o:o + 1024],
                            in_=ag_out[pair][jj][:, 1024:2048])

                fetch_pair(0)

                for fi in range(NFI):
                    wt = wopool.tile([P, H * P], BF16, name=f"{x}wob{fi}",
                                     tag="wob", bufs=16)
                    nc.gpsimd.dma_start(out=wt[:], in_=wo_ext[fi])
                    wo_bf.append(wt)

                out_acc = oapool.tile([P, NFI * NL], F32,
                                      name=f"{x}oacc", tag="oacc")

                def kslice(g, kj):
                    j, u = divmod(kj, 4)
                    o = j * 1024 + (g % 2) * NL + u * P
                    return kt_sb[g // 2][:, o:o + P]

                def vslice(g, kj):
                    j, u = divmod(kj, 4)
                    o = j * 1024 + (g % 2) * NL + u * P
                    return vt_sb[g // 2][:, o:o + P]

                # ---- attention: two 8-head waves ----
                no_sb = [None] * H
                with tc.tile_pool(name=f"{x}ppatt", bufs=1,
                                  space="PSUM") as pp:
                    for pair in range(2):
                        if pair == 1:
                            fetch_pair(1)
                        heads = [2 * pair + (hh % 2) + 4 * (hh // 2)
                                 for hh in range(8)]
                        for hi, h in enumerate(heads):
                            g = h % HKV
                            av_ps = pp.tile([P, NL], F32, name=f"{x}av{h}",
                                            tag="av", bufs=2)
                            den_ps = pp.tile([1, NL], F32, name=f"{x}den{h}",
                                             tag="den", bufs=1)
                            acc = apool.tile([P, NL], F16,
                                             name=f"{x}acc{h}", tag="acc")
                            e_tiles = {}
                            for step in range(9):
                                if step < 8:
                                    s_ps = pp.tile([P, 2 * NL], F32,
                                                   name=f"{x}s{h}_{step}",
                                                   tag="sc", bufs=2)
                                    for i in range(2):
                                        kj = 2 * step + i
                                        nc.tensor.matmul(
                                            s_ps[:, i * NL:(i + 1) * NL],
                                            kslice(g, kj), qr_sb[h][:],
                                            start=True, stop=True)
                                    e_sb = epool.tile([P, 2 * NL], BF16,
                                                      name=f"{x}e{h}_{step}",
                                                      tag="exp")
                                    nc.scalar.activation(
                                        e_sb[:], s_ps[:],
                                        mybir.ActivationFunctionType.Exp,
                                        scale=float(SCALE))
                                    e_tiles[step] = e_sb
                                if step >= 1:
                                    p2 = step - 1
                                    e_sb = e_tiles.pop(p2)
                                    for i in range(2):
                                        kj = 2 * p2 + i
                                        esl = e_sb[:, i * NL:(i + 1) * NL]
                                        nc.tensor.matmul(
                                            av_ps[:], vslice(g, kj), esl,
                                            start=(kj == 0),
                                            stop=(kj == NKJ - 1))
                                        # denominator: accumulate exp
                                        # tiles on DVE (fp16 acc, 2-byte
                                        # fast mode), one final ones-matmul
                                        # does the partition reduction.
                                        if kj == 0:
                                            nc.vector.tensor_copy(
                                                out=acc[:], in_=esl)
                                        else:
                                            nc.vector.tensor_tensor(
                                                out=acc[:], in0=acc[:],
                                                in1=esl,
                                                op=mybir.AluOpType.add)
                            nc.tensor.matmul(den_ps[:], ones_kj[:], acc[:],
                                             start=True, stop=True)
                            recip = fxpool.tile([1, NL], F32,
                                                name=f"{x}rc{h}",
                                                tag="recip", bufs=2)
                            nc.vector.reciprocal(out=recip[:], in_=den_ps[:])
                            bc_sb = fxpool.tile([P, NL], F32,
                                                name=f"{x}bcs{h}",
                                                tag="bcs", bufs=2)
                            nc.gpsimd.partition_broadcast(bc_sb[:],
                                                          recip[:])
                            no = nopool.tile([P, NL], BF16, name=f"{x}no{h}",
                                             tag=f"no{h}")
                            nc.vector.tensor_tensor(out=no[:], in0=av_ps[:],
                                                    in1=bc_sb[:],
                                                    op=mybir.AluOpType.mult)
                            no_sb[h] = no

                        # partial out-projection over this wave's heads:
                        # fills the PE while ACT drains this wave's exps and
                        # the next collective lands.
                        for fi in range(NFI):
                            ps = pp.tile([P, NL], F32,
                                         name=f"{x}po{pair}_{fi}",
                                         tag="pso", bufs=1)
                            for h in heads:
                                nc.tensor.matmul(
                                    ps[:], wo_bf[fi][:, h * P:(h + 1) * P],
                                    no_sb[h][:],
                                    start=(h == heads[0]),
                                    stop=(h == heads[-1]))
                            oa = out_acc[:, fi * NL:(fi + 1) * NL]
                            if pair == 0:
                                nc.vector.tensor_copy(out=oa, in_=ps[:])
                            else:
                                o_sb = opool.tile([P, NL], BF16,
                                                  name=f"{x}o{fi}", tag="osb")
                                nc.vector.scalar_tensor_tensor(
                                    out=o_sb[:], in0=ps[:],
                                    scalar=bias_sb[:, fi:fi + 1],
                                    in1=oa,
                                    op0=mybir.AluOpType.add,
                                    op1=mybir.AluOpType.add)
                                nc.scalar.dma_start(out=outt_ext[fi],
                                                    in_=o_sb[:])

                if stop_after == "attn":
                    nc.sync.dma_start(out=outt_ext[0],
                                      in_=no_sb[0][:].bitcast(F32))
                    return

def build_program(reps=1, single_core=False):
    nc = bacc.Bacc("TRN2", target_bir_lowering=False, debug=False,
                   num_devices=1 if single_core else N_CORES)

    ext = (
        nc.dram_tensor("xt", [ND, P, NL], BF16,
                       kind="ExternalInput").ap(),
        nc.dram_tensor("wqtt", [4, ND, P, NL], BF16,
                       kind="ExternalInput").ap(),
        nc.dram_tensor("wkv01t", [ND, P, NL], BF16,
                       kind="ExternalInput").ap(),
        nc.dram_tensor("wkv23t", [ND, P, NL], BF16,
                       kind="ExternalInput").ap(),
        nc.dram_tensor("wott", [NFI, P, H * P], BF16,
                       kind="ExternalInput").ap(),
        nc.dram_tensor("biast", [P, NFI], F32, kind="ExternalInput").ap(),
        nc.dram_tensor("cost", [P, NL], F32, kind="ExternalInput").ap(),
        nc.dram_tensor("sint", [P, NL], F32, kind="ExternalInput").ap(),
        nc.dram_tensor("outt", [NFI, P, NL], BF16,
                       kind="ExternalOutput").ap(),
    )
    consts = (
        nc.inline_tensor(np.ones((P, 1), np.float16), name="ones_kj_c"),
    )

    with tile.TileContext(nc) as tc:
        for r in range(reps):
            _emit(nc, tc, ext, consts, f"r{r}_" if reps > 1 else "",
                  single_core=single_core)

    nc.compile()
    return nc


def shard_inputs(x, cos, sin, wq, wkv, wo_w, wo_b):
    """Host-side prep: transpose/tile everything into DMA-friendly layouts."""
    x = np.asarray(x, np.float32)
    cos = np.asarray(cos, np.float32)
    sin = np.asarray(sin, np.float32)
    wq = np.asarray(wq, np.float32)
    wkv = np.asarray(wkv, np.float32)
    wo_w = np.asarray(wo_w, np.float32)
    wo_b = np.asarray(wo_b, np.float32)

    wqT = np.ascontiguousarray(wq.T)                      # [d, e]
    # tiles [hg, dt, 128, 512]
    wqtt = np.ascontiguousarray(
        wqT.reshape(ND, P, 4, NL).transpose(2, 0, 1, 3)).astype(
            ml_dtypes.bfloat16)
    wkvT = wkv.T                                          # [d, 1024]
    wk, wv = wkvT[:, 0:512], wkvT[:, 512:1024]
    # per pair: [d, 512] = [K pair (2*128) | V pair (2*128)]
    wkv01 = np.ascontiguousarray(
        np.concatenate([wk[:, 0:256], wv[:, 0:256]], axis=1)
    ).reshape(ND, P, NL).astype(ml_dtypes.bfloat16)
    wkv23 = np.ascontiguousarray(
        np.concatenate([wk[:, 256:512], wv[:, 256:512]], axis=1)
    ).reshape(ND, P, NL).astype(ml_dtypes.bfloat16)
    woT = wo_w.T                                          # [e, f]
    # [fi, a, h, b]: per fi a contiguous [128, 2048] block
    wott = np.ascontiguousarray(
        woT.reshape(H, P, NFI, P).transpose(2, 1, 0, 3)
    ).reshape(NFI, P, H * P).astype(ml_dtypes.bfloat16)
    biast = np.ascontiguousarray(wo_b.reshape(NFI, P).T)  # [128, 16] f32

    in_maps = []
    for c in range(N_CORES):
        b, blk = divmod(c, 4)
        r0 = blk * NL
        xt = np.ascontiguousarray(x[b, r0:r0 + NL, :].T).reshape(
            ND, P, NL).astype(ml_dtypes.bfloat16)
        cosT = cos[0, r0:r0 + NL, 0, :].T                 # [64, n]
        sinT = sin[0, r0:r0 + NL, 0, :].T
        cost = np.ascontiguousarray(np.vstack([cosT, cosT]))   # [128, n]
        sint = np.ascontiguousarray(np.vstack([sinT, sinT]))
        in_maps.append({
            "xt": xt, "wqtt": wqtt, "wkv01t": wkv01, "wkv23t": wkv23,
            "wott": wott, "biast": biast, "cost": cost, "sint": sint,
        })
    return in_maps


def assemble_output(results):
    out = np.empty((B, N, D), np.float32)
    for c in range(N_CORES):
        b, blk = divmod(c, 4)
        r0 = blk * NL
        # outt [NFI, P, NL] -> [d, n] -> transpose
        out[b, r0:r0 + NL, :] = results[c]["outt"].reshape(
            D, NL).astype(np.float32).T
    return out


def get_program(reps=1):
    key = ("nc", reps)
    if key not in _CACHE:
        _CACHE[key] = build_program(reps)
    return _CACHE[key]


def kernel(x, cos, sin, attn_mask, wq, wkv, wo_w, wo_b):
    # attn_mask is all-ones by construction (fill spec); ignored.
    nc = get_program()
    in_maps = shard_inputs(x, cos, sin, wq, wkv, wo_w, wo_b)
    res = bass_utils.run_bass_kernel_spmd(
        nc, in_maps, core_ids=list(range(N_CORES)))
    return assemble_output(res.results)
